# revision 22
# baseline (speedup 1.0000x reference)
"""NT-Xent (SimCLR) loss on 8 Trainium2 NeuronCores.

Full inputs z1, z2: [4096, 256] f32.  z = concat -> [8192, 256], rows
L2-normalized, sim = zn @ zn.T / 0.5 with the diagonal masked out, row
log-softmax, loss = -mean over rows of logp[i, pair(i)].

Sharding: data-parallel over rows.  Every core receives the full z1/z2
(to build the all-column normalized z^T it needs as the matmul moving
operand) plus its own 1024-row block (zrows) and the paired block
(zpair) so the single SPMD program picks its rows purely through input
data.

Device algorithm (per core):
  - sim_ii = 1/T = 2.0 is always the row max, so use a constant shift:
    denom_i = sum_j exp(2*cos_ij - 2) - exp(2*selfdot_i - 2).
    No max pass and no diagonal masking pass needed.
  - rows of sim are produced as  raw_ij = z_i . zn_j  with the row's
    2/||z_i|| folded into the exp's per-partition scale operand, so the
    core's own row block never needs normalizing.
  - exp + row-sum happen in one ScalarE activation over [128, 2048]
    PSUM tiles (accum_out) - ScalarE only ever runs Exp (+ one final Ln).
  - rsqrt built on DVE only (bit-trick seed + 3 Newton steps) so the
    stats ladder never waits on ScalarE.
  - loss_row = 2 + log(denom_i - diag_i) - 2*pos_i.

Engine split: PE transposes+matmuls; ACT Exp/Ln; DVE reduces, rsqrt and
PSUM->SBUF copies; GPSIMD most squares + normalize scaling.  Work
streams in 4 groups of 16 row-tiles (2048 sim columns each); zpair
processing is deferred into the matmul phase where DVE/GPSIMD idle.

Host just slices inputs and averages the 8192 per-row losses.
"""

import numpy as np
from contextlib import ExitStack

import concourse.bass as bass
import concourse.bacc as bacc
import concourse.mybir as mybir
import concourse.tile as tile
from concourse import masks
from concourse.bass import ts
from concourse.bass_utils import run_bass_kernel_spmd

F32 = mybir.dt.float32
F32R = mybir.dt.float32r
I32 = mybir.dt.int32
AF = mybir.ActivationFunctionType
ALU = mybir.AluOpType

P = 128          # partitions
D = 256          # feature dim
N = 4096         # rows per z1 / z2
R = 2 * N        # 8192 total rows
NCORES = 8
RPC = R // NCORES          # 1024 rows per core
NB = RPC // P              # 8 row blocks per core
NT = R // P                # 64 natural tiles of the full z
GROUPS = 4                 # process full z in 4 groups of 16 tiles
TPG = NT // GROUPS         # 16 tiles per group = 2048 sim columns
SCALE = 2.0                # 1/temperature
USE_F32R = True            # fast fp32 matmul path
MM_DT = F32R if USE_F32R else F32


def _dve_rsqrt(nc, scratch, r_view, a_view, magic_view, n, tag, steps=3):
    """r = 1/sqrt(a) entirely on DVE: int bit-trick seed + Newton steps."""
    ri = r_view.bitcast(I32)
    ai = a_view.bitcast(I32)
    nc.vector.tensor_scalar(
        out=ri, in0=ai, scalar1=1, scalar2=None, op0=ALU.arith_shift_right
    )
    nc.vector.tensor_tensor(out=ri, in0=magic_view, in1=ri, op=ALU.subtract)
    for s in range(steps):
        t1 = scratch.tile([P, n], F32, tag=tag, bufs=2, name=f"{tag}_n{s}")
        nc.vector.tensor_tensor(out=t1[:], in0=r_view, in1=r_view, op=ALU.mult)
        nc.vector.tensor_tensor(out=t1[:], in0=t1[:], in1=a_view, op=ALU.mult)
        nc.vector.tensor_scalar(
            out=t1[:], in0=t1[:], scalar1=-0.5, scalar2=1.5,
            op0=ALU.mult, op1=ALU.add,
        )
        nc.vector.tensor_tensor(out=r_view, in0=r_view, in1=t1[:], op=ALU.mult)


def build_nc():
    nc = bacc.Bacc(None, target_bir_lowering=False, debug=False)

    z1 = nc.declare_dram_parameter("z1", [N, D], F32, isOutput=False)
    z2 = nc.declare_dram_parameter("z2", [N, D], F32, isOutput=False)
    zrows = nc.declare_dram_parameter("zrows", [RPC, D], F32, isOutput=False)
    zpair = nc.declare_dram_parameter("zpair", [RPC, D], F32, isOutput=False)
    out = nc.declare_dram_parameter("loss_rows", [NB, P], F32, isOutput=True)

    with tile.TileContext(nc) as tc, ExitStack() as ctx:
        consts = ctx.enter_context(tc.tile_pool(name="consts", bufs=1))
        small = ctx.enter_context(tc.tile_pool(name="small", bufs=1))
        scratch = ctx.enter_context(tc.tile_pool(name="scratch", bufs=2))
        ztiles = ctx.enter_context(tc.tile_pool(name="ztiles", bufs=20))
        znt_pool = ctx.enter_context(tc.tile_pool(name="znt", bufs=1))
        zr_pool = ctx.enter_context(tc.tile_pool(name="zrp", bufs=1))
        psum = ctx.enter_context(
            tc.tile_pool(name="psum", bufs=2, space=bass.MemorySpace.PSUM)
        )
        expout = ctx.enter_context(tc.tile_pool(name="expout", bufs=3))

        identity = consts.tile([P, P], F32)
        masks.make_identity(nc, identity[:])
        negtwo = consts.tile([P, 1], F32)
        nc.gpsimd.memset(negtwo[:], -2.0)
        magic = consts.tile([P, TPG], I32)
        nc.gpsimd.memset(magic[:], 0x5F3759DF)

        # ---- core's own rows: DMA, raw |z|^2, transpose -------------------
        zr = zr_pool.tile([P, NB, D], F32)
        nc.sync.dma_start(zr[:], zrows.rearrange("(b p) d -> p b d", p=P))

        sa = small.tile([P, 2 * NB], F32)   # cols 0..7 |z_row|^2, 8..15 |z_pair|^2
        for b in range(NB):
            sc2 = scratch.tile([P, D], F32, tag="mul_scr", bufs=4, name=f"sqr{b}")
            nc.gpsimd.tensor_tensor(out=sc2[:], in0=zr[:, b], in1=zr[:, b], op=ALU.mult)
            nc.vector.tensor_reduce(
                sa[:, b : b + 1], sc2[:], axis=mybir.AxisListType.X, op=ALU.add
            )
        nc.vector.tensor_scalar_max(sa[:, 0:NB], sa[:, 0:NB], 1e-16)
        rnr = small.tile([P, NB], F32)
        _dve_rsqrt(nc, scratch, rnr[:], sa[:, 0:NB], magic[:, 0:NB], NB, "nwt_r")
        rnr2 = small.tile([P, NB], F32)
        nc.vector.tensor_scalar_mul(rnr2[:], rnr[:], SCALE)

        # transpose raw row block -> zrawT [128, 2(k), 1024]
        zrawT = zr_pool.tile([P, 2, RPC], MM_DT)
        ptr = psum.tile([P, 2, 2, 512], F32, tag="ps", name="ptr")
        for half in range(2):
            for j in range(4):
                b = half * 4 + j
                for k in range(2):
                    nc.tensor.transpose(
                        ptr[:, k, half, ts(j, P)], zr[:, b, ts(k, P)], identity[:]
                    )
        nc.vector.tensor_copy(
            zrawT[:].rearrange("p k (h c) -> p k h c", c=512), ptr[:]
        )

        # ---- full z: stream 4 groups of 16 tiles --------------------------
        znt = [
            znt_pool.tile([P, 2, 4, 512], MM_DT, tag=f"znt{g}", name=f"znt{g}")
            for g in range(GROUPS)
        ]  # [k, chunk-in-group, col]
        sqf = small.tile([P, NT], F32)
        rnf = small.tile([P, NT], F32)
        denoms = small.tile([P, NB, GROUPS], F32)

        for grp in range(GROUPS):
            t0 = grp * TPG
            gtiles = []
            for ti, t in enumerate(range(t0, t0 + TPG)):
                zt = ztiles.tile([P, D], F32, tag="zt", bufs=20, name=f"zt{t}")
                src = z1 if t < NT // 2 else z2
                nc.sync.dma_start(zt[:], src[ts(t % (NT // 2), P), :])
                sc = scratch.tile([P, D], F32, tag="mul_scr", bufs=4, name=f"sq{t}")
                sq_eng = nc.vector if ti % 4 == 0 else nc.gpsimd
                sq_eng.tensor_tensor(out=sc[:], in0=zt[:], in1=zt[:], op=ALU.mult)
                nc.vector.tensor_reduce(
                    sqf[:, t : t + 1], sc[:], axis=mybir.AxisListType.X, op=ALU.add
                )
                gtiles.append(zt)

            gs = slice(t0, t0 + TPG)
            nc.vector.tensor_scalar_max(sqf[:, gs], sqf[:, gs], 1e-16)
            _dve_rsqrt(nc, scratch, rnf[:, gs], sqf[:, gs], magic[:], TPG, "nwt_g")

            # normalize + transpose the 16 tiles into the group's 4 chunks
            for hp in range(2):  # chunk pairs (q=0,1) then (q=2,3)
                pt = psum.tile([P, 2, 2, 512], F32, tag="ps", name=f"pt{grp}_{hp}")
                for jj in range(8):
                    ti = hp * 8 + jj
                    t = t0 + ti
                    q_sub, j4 = jj // 4, jj % 4
                    zt = gtiles[ti]
                    nrm_eng = nc.vector if ti % 4 == 0 else nc.gpsimd
                    nrm_eng.tensor_scalar_mul(zt[:], zt[:], rnf[:, t : t + 1])
                    for k in range(2):
                        nc.tensor.transpose(
                            pt[:, k, q_sub, ts(j4, P)], zt[:, ts(k, P)], identity[:]
                        )
                nc.vector.tensor_copy(znt[grp][:, :, 2 * hp : 2 * hp + 2, :], pt[:])

            # ---- matmul + exp over this group's 2048 columns --------------
            for b in range(NB):
                pm = psum.tile([P, 4, 512], F32, tag="ps", name=f"pm{grp}_{b}")
                for q in range(4):
                    for k in range(2):
                        nc.tensor.matmul(
                            pm[:, q, :],
                            zrawT[:, k, ts(b, P)],
                            znt[grp][:, k, q, :],
                            start=(k == 0),
                            stop=(k == 1),
                        )
                eo = expout.tile(
                    [P, 4, 512], F32, tag="eo", bufs=3, name=f"eo{grp}_{b}"
                )
                nc.scalar.activation(
                    eo[:], pm[:], AF.Exp,
                    bias=negtwo[:], scale=rnr2[:, b : b + 1],
                    accum_out=denoms[:, b, grp : grp + 1],
                )

            if grp == 0:
                # paired rows: deferred here so it overlaps the matmul phase
                zp = zr_pool.tile([P, NB, D], F32)
                nc.sync.dma_start(zp[:], zpair.rearrange("(b p) d -> p b d", p=P))
                rawpos = small.tile([P, NB], F32)
                for b in range(NB):
                    sc3 = scratch.tile([P, D], F32, tag="mul_scr", bufs=4,
                                       name=f"sqp{b}")
                    nc.gpsimd.tensor_tensor(
                        out=sc3[:], in0=zp[:, b], in1=zp[:, b], op=ALU.mult
                    )
                    nc.vector.tensor_reduce(
                        sa[:, NB + b : NB + b + 1], sc3[:],
                        axis=mybir.AxisListType.X, op=ALU.add,
                    )
                    sc4 = scratch.tile([P, D], F32, tag="mul_scr", bufs=4,
                                       name=f"pos{b}")
                    nc.gpsimd.tensor_tensor(
                        out=sc4[:], in0=zr[:, b], in1=zp[:, b], op=ALU.mult
                    )
                    nc.vector.tensor_reduce(
                        rawpos[:, b : b + 1], sc4[:],
                        axis=mybir.AxisListType.X, op=ALU.add,
                    )
                nc.vector.tensor_scalar_max(sa[:, NB:], sa[:, NB:], 1e-16)
                rnp = small.tile([P, NB], F32)
                _dve_rsqrt(nc, scratch, rnp[:], sa[:, NB:], magic[:, 0:NB], NB,
                           "nwt_p")
                # diag term exp(2*selfdot_unit - 2), selfdot = |z|^2 * rnr^2
                sd = small.tile([P, NB], F32)
                nc.vector.tensor_tensor(out=sd[:], in0=sa[:, 0:NB], in1=rnr[:],
                                        op=ALU.mult)
                nc.vector.tensor_tensor(out=sd[:], in0=sd[:], in1=rnr[:],
                                        op=ALU.mult)
                diag = small.tile([P, NB], F32)
                nc.scalar.activation(diag[:], sd[:], AF.Exp, bias=negtwo[:],
                                     scale=SCALE)
                # pos_unit = rawpos * rn_row * rn_pair
                posx = small.tile([P, NB], F32)
                nc.vector.tensor_tensor(out=posx[:], in0=rawpos[:], in1=rnr[:],
                                        op=ALU.mult)
                nc.vector.tensor_tensor(out=posx[:], in0=posx[:], in1=rnp[:],
                                        op=ALU.mult)

        # ---- epilogue: per-row loss ---------------------------------------
        denom = small.tile([P, NB], F32)
        nc.vector.tensor_reduce(
            denom[:], denoms[:], axis=mybir.AxisListType.X, op=ALU.add
        )
        nc.vector.tensor_tensor(out=denom[:], in0=denom[:], in1=diag[:],
                                op=ALU.subtract)
        logd = small.tile([P, NB], F32)
        nc.scalar.activation(logd[:], denom[:], AF.Ln)
        loss = small.tile([P, NB], F32)
        nc.vector.tensor_scalar_mul(loss[:], posx[:], -2.0)
        nc.vector.tensor_tensor(out=loss[:], in0=loss[:], in1=logd[:], op=ALU.add)
        nc.vector.tensor_scalar_add(loss[:], loss[:], 2.0)

        # transpose [128, 8] -> [8, 128] so the output DMA is contiguous
        pl = psum.tile([P, 4, 512], F32, tag="ps")
        nc.tensor.transpose(pl[0:NB, 0, 0:P], loss[:], identity[:])
        outsb = small.tile([NB, P], F32)
        nc.vector.tensor_copy(outsb[:], pl[0:NB, 0, 0:P])
        nc.sync.dma_start(out[:, :], outsb[:])

    nc.compile()
    return nc


_NC = None


def _get_nc():
    global _NC
    if _NC is None:
        _NC = build_nc()
    return _NC


def _in_maps(z1, z2):
    z1 = np.ascontiguousarray(z1, dtype=np.float32)
    z2 = np.ascontiguousarray(z2, dtype=np.float32)
    z = np.concatenate([z1, z2], axis=0)
    maps = []
    for c in range(NCORES):
        lo = c * RPC
        plo = (lo + N) % R
        maps.append(
            {
                "z1": z1,
                "z2": z2,
                "zrows": np.ascontiguousarray(z[lo : lo + RPC]),
                "zpair": np.ascontiguousarray(z[plo : plo + RPC]),
            }
        )
    return maps


def run(z1, z2, trace=False, **kwargs):
    nc = _get_nc()
    res = run_bass_kernel_spmd(
        nc, _in_maps(z1, z2), list(range(NCORES)), trace=trace, **kwargs
    )
    rows = np.concatenate(
        [np.asarray(res.results[c]["loss_rows"]).reshape(-1) for c in range(NCORES)]
    )
    return np.float32(rows.mean()), res


def kernel(z1, z2):
    loss, _ = run(z1, z2)
    return loss


# revision 24
# speedup vs baseline: 1.7021x; 1.7021x over previous
"""NT-Xent (SimCLR) loss on 8 Trainium2 NeuronCores.

Full inputs z1, z2: [4096, 256] f32.  z = concat -> [8192, 256], rows
L2-normalized, sim = zn @ zn.T / 0.5 with the diagonal masked out, row
log-softmax, loss = -mean over rows of logp[i, pair(i)].

Sharding: data-parallel over rows.  Every core receives the full z1/z2
(to build the all-column normalized z^T it needs as the matmul moving
operand) plus its own 1024-row block (zrows) and the paired block
(zpair) so the single SPMD program picks its rows purely through input
data.

Device algorithm (per core):
  - sim_ii = 1/T = 2.0 is always the row max, so use a constant shift:
    denom_i = sum_j exp(2*cos_ij - 2) - exp(2*selfdot_i - 2).
    No max pass and no diagonal masking pass needed.
  - rows of sim are produced as  raw_ij = z_i . zn_j  with the row's
    2/||z_i|| folded into the exp's per-partition scale operand, so the
    core's own row block never needs normalizing.
  - exp + row-sum happen in one ScalarE activation over [128, 2048]
    PSUM tiles (accum_out) - ScalarE only ever runs Exp (+ one final Ln).
  - rsqrt built on DVE only (bit-trick seed + 3 Newton steps) so the
    stats ladder never waits on ScalarE.
  - loss_row = 2 + log(denom_i - diag_i) - 2*pos_i.

Engine split: PE transposes+matmuls; ACT Exp/Ln; DVE reduces, rsqrt and
PSUM->SBUF copies; GPSIMD most squares + normalize scaling.  Work
streams in 4 groups of 16 row-tiles (2048 sim columns each); zpair
processing is deferred into the matmul phase where DVE/GPSIMD idle.

Host just slices inputs and averages the 8192 per-row losses.
"""

import numpy as np
from contextlib import ExitStack

import concourse.bass as bass
import concourse.bacc as bacc
import concourse.mybir as mybir
import concourse.tile as tile
from concourse import masks
from concourse.bass import ts
from concourse.bass_utils import run_bass_kernel_spmd

F32 = mybir.dt.float32
F32R = mybir.dt.float32r
I32 = mybir.dt.int32
AF = mybir.ActivationFunctionType
ALU = mybir.AluOpType

P = 128          # partitions
D = 256          # feature dim
N = 4096         # rows per z1 / z2
R = 2 * N        # 8192 total rows
NCORES = 8
RPC = R // NCORES          # 1024 rows per core
NB = RPC // P              # 8 row blocks per core
NT = R // P                # 64 natural tiles of the full z
GROUPS = 4                 # process full z in 4 groups of 16 tiles
TPG = NT // GROUPS         # 16 tiles per group = 2048 sim columns
SCALE = 2.0                # 1/temperature
USE_F32R = True            # fast fp32 matmul path
MM_DT = F32R if USE_F32R else F32


def _dve_rsqrt(nc, scratch, r_view, a_view, magic_view, n, tag, steps=3):
    """r = 1/sqrt(a) entirely on DVE: int bit-trick seed + Newton steps."""
    ri = r_view.bitcast(I32)
    ai = a_view.bitcast(I32)
    nc.vector.tensor_scalar(
        out=ri, in0=ai, scalar1=1, scalar2=None, op0=ALU.arith_shift_right
    )
    nc.vector.tensor_tensor(out=ri, in0=magic_view, in1=ri, op=ALU.subtract)
    for s in range(steps):
        t1 = scratch.tile([P, n], F32, tag=tag, bufs=2, name=f"{tag}_n{s}")
        nc.vector.tensor_tensor(out=t1[:], in0=r_view, in1=r_view, op=ALU.mult)
        nc.vector.tensor_tensor(out=t1[:], in0=t1[:], in1=a_view, op=ALU.mult)
        nc.vector.tensor_scalar(
            out=t1[:], in0=t1[:], scalar1=-0.5, scalar2=1.5,
            op0=ALU.mult, op1=ALU.add,
        )
        nc.vector.tensor_tensor(out=r_view, in0=r_view, in1=t1[:], op=ALU.mult)


def build_nc(loop_n=None):
    nc = bacc.Bacc(None, target_bir_lowering=False, debug=False)

    z1 = nc.declare_dram_parameter("z1", [N, D], F32, isOutput=False)
    z2 = nc.declare_dram_parameter("z2", [N, D], F32, isOutput=False)
    zrows = nc.declare_dram_parameter("zrows", [RPC, D], F32, isOutput=False)
    zpair = nc.declare_dram_parameter("zpair", [RPC, D], F32, isOutput=False)
    out = nc.declare_dram_parameter("loss_rows", [NB, P], F32, isOutput=True)

    with tile.TileContext(nc) as tc, ExitStack() as ctx:
        consts = ctx.enter_context(tc.tile_pool(name="consts", bufs=1))
        small = ctx.enter_context(tc.tile_pool(name="small", bufs=1))
        scratch = ctx.enter_context(tc.tile_pool(name="scratch", bufs=2))
        ztiles = ctx.enter_context(tc.tile_pool(name="ztiles", bufs=20))
        znt_pool = ctx.enter_context(tc.tile_pool(name="znt", bufs=1))
        zr_pool = ctx.enter_context(tc.tile_pool(name="zrp", bufs=1))
        psum = ctx.enter_context(
            tc.tile_pool(name="psum", bufs=2, space=bass.MemorySpace.PSUM)
        )
        expout = ctx.enter_context(tc.tile_pool(name="expout", bufs=3))

        identity = consts.tile([P, P], F32)
        masks.make_identity(nc, identity[:])
        negtwo = consts.tile([P, 1], F32)
        nc.gpsimd.memset(negtwo[:], -2.0)
        magic = consts.tile([P, TPG], I32)
        nc.gpsimd.memset(magic[:], 0x5F3759DF)

        loop_cm = tc.For_i(0, loop_n, 1) if loop_n else ExitStack()
        ctx.enter_context(loop_cm)

        # ---- core's own rows: DMA, raw |z|^2, transpose -------------------
        zr = zr_pool.tile([P, NB, D], F32)
        nc.sync.dma_start(zr[:], zrows.rearrange("(b p) d -> p b d", p=P))

        sa = small.tile([P, 2 * NB], F32)   # cols 0..7 |z_row|^2, 8..15 |z_pair|^2
        for b in range(NB):
            sc2 = scratch.tile([P, D], F32, tag="mul_scr", bufs=4, name=f"sqr{b}")
            nc.gpsimd.tensor_tensor(out=sc2[:], in0=zr[:, b], in1=zr[:, b], op=ALU.mult)
            nc.vector.tensor_reduce(
                sa[:, b : b + 1], sc2[:], axis=mybir.AxisListType.X, op=ALU.add
            )
        nc.vector.tensor_scalar_max(sa[:, 0:NB], sa[:, 0:NB], 1e-16)
        rnr = small.tile([P, NB], F32)
        _dve_rsqrt(nc, scratch, rnr[:], sa[:, 0:NB], magic[:, 0:NB], NB, "nwt_r")
        rnr2 = small.tile([P, NB], F32)
        nc.vector.tensor_scalar_mul(rnr2[:], rnr[:], SCALE)

        # transpose raw row block -> zrawT [128, 2(k), 1024]
        zrawT = zr_pool.tile([P, 2, RPC], MM_DT)
        ptr = psum.tile([P, 2, 2, 512], F32, tag="ps", name="ptr")
        for half in range(2):
            for j in range(4):
                b = half * 4 + j
                for k in range(2):
                    nc.tensor.transpose(
                        ptr[:, k, half, ts(j, P)], zr[:, b, ts(k, P)], identity[:]
                    )
        nc.vector.tensor_copy(
            zrawT[:].rearrange("p k (h c) -> p k h c", c=512), ptr[:]
        )

        # ---- full z: stream 4 groups of 16 tiles --------------------------
        znt = [
            znt_pool.tile([P, 2, 4, 512], MM_DT, tag=f"znt{g}", name=f"znt{g}")
            for g in range(GROUPS)
        ]  # [k, chunk-in-group, col]
        sqf = small.tile([P, NT], F32)
        rnf = small.tile([P, NT], F32)
        denoms = small.tile([P, NB, GROUPS], F32)

        for grp in range(GROUPS):
            t0 = grp * TPG
            gtiles = []
            for ti, t in enumerate(range(t0, t0 + TPG)):
                zt = ztiles.tile([P, D], F32, tag="zt", bufs=20, name=f"zt{t}")
                src = z1 if t < NT // 2 else z2
                nc.sync.dma_start(zt[:], src[ts(t % (NT // 2), P), :])
                sc = scratch.tile([P, D], F32, tag="mul_scr", bufs=4, name=f"sq{t}")
                sq_eng = nc.vector if ti % 4 == 0 else nc.gpsimd
                sq_eng.tensor_tensor(out=sc[:], in0=zt[:], in1=zt[:], op=ALU.mult)
                nc.vector.tensor_reduce(
                    sqf[:, t : t + 1], sc[:], axis=mybir.AxisListType.X, op=ALU.add
                )
                gtiles.append(zt)

            gs = slice(t0, t0 + TPG)
            nc.vector.tensor_scalar_max(sqf[:, gs], sqf[:, gs], 1e-16)
            _dve_rsqrt(nc, scratch, rnf[:, gs], sqf[:, gs], magic[:], TPG, "nwt_g")

            # normalize + transpose the 16 tiles into the group's 4 chunks
            for hp in range(2):  # chunk pairs (q=0,1) then (q=2,3)
                pt = psum.tile([P, 2, 2, 512], F32, tag="ps", name=f"pt{grp}_{hp}")
                for jj in range(8):
                    ti = hp * 8 + jj
                    t = t0 + ti
                    q_sub, j4 = jj // 4, jj % 4
                    zt = gtiles[ti]
                    nrm_eng = nc.vector if ti % 4 == 0 else nc.gpsimd
                    nrm_eng.tensor_scalar_mul(zt[:], zt[:], rnf[:, t : t + 1])
                    for k in range(2):
                        nc.tensor.transpose(
                            pt[:, k, q_sub, ts(j4, P)], zt[:, ts(k, P)], identity[:]
                        )
                nc.vector.tensor_copy(znt[grp][:, :, 2 * hp : 2 * hp + 2, :], pt[:])

            # ---- matmul + exp over this group's 2048 columns --------------
            for b in range(NB):
                pm = psum.tile([P, 4, 512], F32, tag="ps", name=f"pm{grp}_{b}")
                for q in range(4):
                    for k in range(2):
                        nc.tensor.matmul(
                            pm[:, q, :],
                            zrawT[:, k, ts(b, P)],
                            znt[grp][:, k, q, :],
                            start=(k == 0),
                            stop=(k == 1),
                        )
                eo = expout.tile(
                    [P, 4, 512], F32, tag="eo", bufs=3, name=f"eo{grp}_{b}"
                )
                nc.scalar.activation(
                    eo[:], pm[:], AF.Exp,
                    bias=negtwo[:], scale=rnr2[:, b : b + 1],
                    accum_out=denoms[:, b, grp : grp + 1],
                )

            if grp == 0:
                # paired rows: deferred here so it overlaps the matmul phase
                zp = zr_pool.tile([P, NB, D], F32)
                nc.sync.dma_start(zp[:], zpair.rearrange("(b p) d -> p b d", p=P))
                rawpos = small.tile([P, NB], F32)
                for b in range(NB):
                    sc3 = scratch.tile([P, D], F32, tag="mul_scr", bufs=4,
                                       name=f"sqp{b}")
                    nc.gpsimd.tensor_tensor(
                        out=sc3[:], in0=zp[:, b], in1=zp[:, b], op=ALU.mult
                    )
                    nc.vector.tensor_reduce(
                        sa[:, NB + b : NB + b + 1], sc3[:],
                        axis=mybir.AxisListType.X, op=ALU.add,
                    )
                    sc4 = scratch.tile([P, D], F32, tag="mul_scr", bufs=4,
                                       name=f"pos{b}")
                    nc.gpsimd.tensor_tensor(
                        out=sc4[:], in0=zr[:, b], in1=zp[:, b], op=ALU.mult
                    )
                    nc.vector.tensor_reduce(
                        rawpos[:, b : b + 1], sc4[:],
                        axis=mybir.AxisListType.X, op=ALU.add,
                    )
                nc.vector.tensor_scalar_max(sa[:, NB:], sa[:, NB:], 1e-16)
                rnp = small.tile([P, NB], F32)
                _dve_rsqrt(nc, scratch, rnp[:], sa[:, NB:], magic[:, 0:NB], NB,
                           "nwt_p")
                # diag term exp(2*selfdot_unit - 2), selfdot = |z|^2 * rnr^2
                sd = small.tile([P, NB], F32)
                nc.vector.tensor_tensor(out=sd[:], in0=sa[:, 0:NB], in1=rnr[:],
                                        op=ALU.mult)
                nc.vector.tensor_tensor(out=sd[:], in0=sd[:], in1=rnr[:],
                                        op=ALU.mult)
                diag = small.tile([P, NB], F32)
                nc.scalar.activation(diag[:], sd[:], AF.Exp, bias=negtwo[:],
                                     scale=SCALE)
                # pos_unit = rawpos * rn_row * rn_pair
                posx = small.tile([P, NB], F32)
                nc.vector.tensor_tensor(out=posx[:], in0=rawpos[:], in1=rnr[:],
                                        op=ALU.mult)
                nc.vector.tensor_tensor(out=posx[:], in0=posx[:], in1=rnp[:],
                                        op=ALU.mult)

        # ---- epilogue: per-row loss ---------------------------------------
        denom = small.tile([P, NB], F32)
        nc.vector.tensor_reduce(
            denom[:], denoms[:], axis=mybir.AxisListType.X, op=ALU.add
        )
        nc.vector.tensor_tensor(out=denom[:], in0=denom[:], in1=diag[:],
                                op=ALU.subtract)
        logd = small.tile([P, NB], F32)
        nc.scalar.activation(logd[:], denom[:], AF.Ln)
        loss = small.tile([P, NB], F32)
        nc.vector.tensor_scalar_mul(loss[:], posx[:], -2.0)
        nc.vector.tensor_tensor(out=loss[:], in0=loss[:], in1=logd[:], op=ALU.add)
        nc.vector.tensor_scalar_add(loss[:], loss[:], 2.0)

        # transpose [128, 8] -> [8, 128] so the output DMA is contiguous
        pl = psum.tile([P, 4, 512], F32, tag="ps")
        nc.tensor.transpose(pl[0:NB, 0, 0:P], loss[:], identity[:])
        outsb = small.tile([NB, P], F32)
        nc.vector.tensor_copy(outsb[:], pl[0:NB, 0, 0:P])
        nc.sync.dma_start(out[:, :], outsb[:])

    nc.compile()
    return nc


_NC = None


def _get_nc():
    global _NC
    if _NC is None:
        _NC = build_nc()
    return _NC


def _in_maps(z1, z2):
    z1 = np.ascontiguousarray(z1, dtype=np.float32)
    z2 = np.ascontiguousarray(z2, dtype=np.float32)
    z = np.concatenate([z1, z2], axis=0)
    maps = []
    for c in range(NCORES):
        lo = c * RPC
        plo = (lo + N) % R
        maps.append(
            {
                "z1": z1,
                "z2": z2,
                "zrows": np.ascontiguousarray(z[lo : lo + RPC]),
                "zpair": np.ascontiguousarray(z[plo : plo + RPC]),
            }
        )
    return maps


def run(z1, z2, trace=False, **kwargs):
    nc = _get_nc()
    res = run_bass_kernel_spmd(
        nc, _in_maps(z1, z2), list(range(NCORES)), trace=trace, **kwargs
    )
    rows = np.concatenate(
        [np.asarray(res.results[c]["loss_rows"]).reshape(-1) for c in range(NCORES)]
    )
    return np.float32(rows.mean()), res


def kernel(z1, z2):
    loss, _ = run(z1, z2)
    return loss


# revision 39
# speedup vs baseline: 3.3046x; 1.9415x over previous
"""NT-Xent (SimCLR) loss on 8 Trainium2 NeuronCores.

Full inputs z1, z2: [4096, 256] f32.  z = concat -> [8192, 256], rows
L2-normalized, sim = zn @ zn.T / 0.5 with the diagonal masked out, row
log-softmax, loss = -mean over rows of logp[i, pair(i)].

Sharding: data-parallel over rows.  Every core receives the full z1/z2
(to build the all-column normalized z^T it needs as the matmul moving
operand) plus its own 1024-row block (zrows) and the paired block
(zpair), so the single SPMD program picks its rows purely through input
data.  The host only slices inputs and averages the 8192 row losses.

Device algorithm (per core):
  - sim_ii = 1/T = 2.0 is always the row max, so a constant shift works:
    denom_i = sum_j exp(2*cos_ij - 2) - exp(2*selfdot_i - 2).
    No max pass and no diagonal masking pass needed.
  - Row blocks stay un-normalized; the row's 2/||z_i|| is folded into
    the exp's per-partition scale operand.
  - exp + row-sum happen in one ScalarE activation over [128, 2048]
    PSUM tiles (accum_out); ScalarE only ever runs Exp (+ one final Ln).
  - rsqrt on DVE only (bit-trick seed + Newton) - no table switches.
  - loss_row = 2 + log(denom_i - diag_i) - 2*pos_i.

Because softmax denominators sum over ALL columns, the column order of
z^T is irrelevant, and row order only permutes the output rows (the
host takes a mean).  That freedom lets every DMA use the row-permuted
AP "(p r) d -> p r d", giving 16 KB-contiguous descriptors per
partition (max DMA bandwidth), and lets stats be batched: per 16-tile
group ONE square, ONE reduce, ONE broadcast-normalize - per-op and
cross-engine-handoff overheads dominate on this hardware, so op count
is everything.
"""

import numpy as np
from contextlib import ExitStack

import concourse.bass as bass
import concourse.bacc as bacc
import concourse.mybir as mybir
import concourse.tile as tile
from concourse import masks
from concourse.bass import ts
from concourse.bass_utils import run_bass_kernel_spmd

F32 = mybir.dt.float32
F32R = mybir.dt.float32r
I32 = mybir.dt.int32
AF = mybir.ActivationFunctionType
ALU = mybir.AluOpType

P = 128          # partitions
D = 256          # feature dim
N = 4096         # rows per z1 / z2
R = 2 * N        # 8192 total rows
NCORES = 8
RPC = R // NCORES          # 1024 rows per core
NB = RPC // P              # 8 row blocks per core
NT = R // P                # 64 natural tiles of the full z
GROUPS = 4                 # process full z in 4 groups of 16 tiles
TPG = NT // GROUPS         # 16 tiles per group = 2048 sim columns
SCALE = 2.0                # 1/temperature
BF16 = mybir.dt.bfloat16
SQ_ON_ACT = True           # where the 6 big squares run
MM_DT = BF16               # matmul operand dtype (bf16: N=1024 moving, cheap
                           # transposes/copies; one rounding of zn -> ~4e-7
                           # end-to-end loss error, same as f32r on this HW)


def _dve_rsqrt(nc, scratch, r_view, a_view, magic_view, n, tag, steps=2):
    """r = 1/sqrt(a) entirely on DVE: int bit-trick seed + Newton steps."""
    ri = r_view.bitcast(I32)
    ai = a_view.bitcast(I32)
    nc.vector.tensor_scalar(
        out=ri, in0=ai, scalar1=1, scalar2=None, op0=ALU.arith_shift_right
    )
    nc.vector.tensor_tensor(out=ri, in0=magic_view, in1=ri, op=ALU.subtract)
    for s in range(steps):
        t1 = scratch.tile([P, n], F32, tag=tag, bufs=2, name=f"{tag}_n{s}")
        nc.vector.tensor_tensor(out=t1[:], in0=r_view, in1=r_view, op=ALU.mult)
        nc.vector.tensor_tensor(out=t1[:], in0=t1[:], in1=a_view, op=ALU.mult)
        nc.vector.tensor_scalar(
            out=t1[:], in0=t1[:], scalar1=-0.5, scalar2=1.5,
            op0=ALU.mult, op1=ALU.add,
        )
        nc.vector.tensor_tensor(out=r_view, in0=r_view, in1=t1[:], op=ALU.mult)


def build_nc(loop_n=None, stage="full"):
    # stage: ablation for timing - "dma", "stats", "trans", "mm", "full"
    S = {"dma": 0, "stats": 1, "trans": 2, "mm": 3, "full": 4}[stage]
    nc = bacc.Bacc(None, target_bir_lowering=False, debug=False)

    z1 = nc.declare_dram_parameter("z1", [N, D], F32, isOutput=False)
    z2 = nc.declare_dram_parameter("z2", [N, D], F32, isOutput=False)
    zrows = nc.declare_dram_parameter("zrows", [RPC, D], F32, isOutput=False)
    zpair = nc.declare_dram_parameter("zpair", [RPC, D], F32, isOutput=False)
    out = nc.declare_dram_parameter("loss_rows", [NB, P], F32, isOutput=True)

    with tile.TileContext(nc) as tc, ExitStack() as ctx:
        consts = ctx.enter_context(tc.tile_pool(name="consts", bufs=1))
        small = ctx.enter_context(tc.tile_pool(name="small", bufs=1))
        scratch = ctx.enter_context(tc.tile_pool(name="scratch", bufs=2))
        zgp = ctx.enter_context(tc.tile_pool(name="zgp", bufs=2))
        znt_pool = ctx.enter_context(tc.tile_pool(name="znt", bufs=1))
        zr_pool = ctx.enter_context(tc.tile_pool(name="zrp", bufs=1))
        psum = ctx.enter_context(
            tc.tile_pool(name="psum", bufs=2, space=bass.MemorySpace.PSUM)
        )
        expout = ctx.enter_context(tc.tile_pool(name="expout", bufs=2))

        identity = consts.tile([P, P], F32)
        masks.make_identity(nc, identity[:])
        negtwo = consts.tile([P, 1], F32)
        nc.gpsimd.memset(negtwo[:], -2.0)
        magic = consts.tile([P, TPG], I32)
        nc.gpsimd.memset(magic[:], 0x5F3759DF)

        loop_cm = tc.For_i(0, loop_n, 1) if loop_n else ExitStack()
        ctx.enter_context(loop_cm)

        # group-0 slab first so its stats ladder starts right away
        zg0 = zgp.tile([P, TPG, D], F32, tag="zg", bufs=2, name="zg0")
        nc.sync.dma_start(
            zg0[:], z1[0 : TPG * P, :].rearrange("(p r) d -> p r d", r=TPG)
        )

        # ---- core's own rows (row p*8+r lives at [p, r, :]) ---------------
        zr = zr_pool.tile([P, NB, D], F32)
        nc.sync.dma_start(zr[:], zrows.rearrange("(p r) d -> p r d", r=NB))

        sa = small.tile([P, 2 * NB], F32)  # |z_row|^2 cols 0..7, |z_pair|^2 8..15
        rnr2 = small.tile([P, NB], F32)
        zp = zr_pool.tile([P, NB, D], F32)
        nc.sync.dma_start(zp[:], zpair.rearrange("(p r) d -> p r d", r=NB))
        rn2 = small.tile([P, 2 * NB], F32)
        rawpos = small.tile([P, NB], F32)
        if S >= 1:
            sq = scratch.tile([P, NB, D], F32, tag="mul", bufs=2, name="sqzr")
            if SQ_ON_ACT:
                nc.scalar.activation(sq[:], zr[:], AF.Square)
            else:
                nc.gpsimd.tensor_tensor(out=sq[:], in0=zr[:], in1=zr[:],
                                        op=ALU.mult)
            nc.vector.tensor_reduce(
                sa[:, 0:NB], sq[:], axis=mybir.AxisListType.X, op=ALU.add
            )
            sqp = scratch.tile([P, NB, D], F32, tag="mul", bufs=2, name="sqzp")
            if SQ_ON_ACT:
                nc.scalar.activation(sqp[:], zp[:], AF.Square)
            else:
                nc.vector.tensor_tensor(out=sqp[:], in0=zp[:], in1=zp[:],
                                        op=ALU.mult)
            nc.vector.tensor_reduce(
                sa[:, NB:], sqp[:], axis=mybir.AxisListType.X, op=ALU.add
            )
            posm = scratch.tile([P, NB, D], F32, tag="mul", bufs=2, name="posm")
            nc.gpsimd.tensor_tensor(out=posm[:], in0=zr[:], in1=zp[:], op=ALU.mult)
            nc.vector.tensor_reduce(
                rawpos[:], posm[:], axis=mybir.AxisListType.X, op=ALU.add
            )
            nc.vector.tensor_scalar_max(sa[:], sa[:], 1e-16)
            _dve_rsqrt(nc, scratch, rn2[:], sa[:], magic[:, 0 : 2 * NB], 2 * NB,
                       "nwt_r")
            nc.vector.tensor_scalar_mul(rnr2[:], rn2[:, 0:NB], SCALE)

        # transpose raw row block -> zrawT [128, 2(k), 1024]
        zrawT = zr_pool.tile([P, 2, RPC], MM_DT)
        if S >= 2:
            ptr = psum.tile([P, 2, 2, 512], F32, tag="ps", name="ptr")
            for half in range(2):
                for j in range(4):
                    b = half * 4 + j
                    for k in range(2):
                        nc.tensor.transpose(
                            ptr[:, k, half, ts(j, P)], zr[:, b, ts(k, P)],
                            identity[:],
                        )
            nc.vector.tensor_copy(
                zrawT[:].rearrange("p k (h c) -> p k h c", c=512), ptr[:]
            )

        # ---- full z: 4 groups of 16 row-tiles -----------------------------
        znt = [
            znt_pool.tile([P, 2, 4, 512], MM_DT, tag=f"znt{g}", name=f"znt{g}")
            for g in range(GROUPS)
        ]  # [k, chunk-in-group, col]
        sqf = small.tile([P, NT], F32)
        rnf = small.tile([P, NT], F32)
        denoms = small.tile([P, NB, GROUPS], F32)

        for grp in range(GROUPS):
            t0 = grp * TPG
            gs = slice(t0, t0 + TPG)
            src = z1 if grp < GROUPS // 2 else z2
            row0 = (grp % (GROUPS // 2)) * (TPG * P)
            if grp == 0:
                zg = zg0
            else:
                zg = zgp.tile([P, TPG, D], F32, tag="zg", bufs=2, name=f"zg{grp}")
                nc.sync.dma_start(
                    zg[:],
                    src[row0 : row0 + TPG * P, :].rearrange(
                        "(p r) d -> p r d", r=TPG
                    ),
                )
            if S >= 1:
                sq = scratch.tile([P, TPG, D], F32, tag="mul", bufs=2,
                                  name=f"sqg{grp}")
                if SQ_ON_ACT:
                    nc.scalar.activation(sq[:], zg[:], AF.Square)
                else:
                    sq_eng = nc.gpsimd if grp % 2 == 0 else nc.vector
                    sq_eng.tensor_tensor(out=sq[:], in0=zg[:], in1=zg[:],
                                         op=ALU.mult)
                nc.vector.tensor_reduce(
                    sqf[:, gs], sq[:], axis=mybir.AxisListType.X, op=ALU.add
                )
                nc.vector.tensor_scalar_max(sqf[:, gs], sqf[:, gs], 1e-16)
                _dve_rsqrt(nc, scratch, rnf[:, gs], sqf[:, gs], magic[:], TPG,
                           "nwt_g")
                nc.gpsimd.tensor_tensor(
                    out=zg[:], in0=zg[:],
                    in1=rnf[:, gs].to_broadcast((P, TPG, D)), op=ALU.mult,
                )

            if S >= 2:
                for hp in range(2):  # chunk pairs (q=0,1) then (q=2,3)
                    pt = psum.tile([P, 2, 2, 512], F32, tag="ps",
                                   name=f"pt{grp}_{hp}")
                    for jj in range(8):
                        ti = hp * 8 + jj
                        q_sub, j4 = jj // 4, jj % 4
                        for k in range(2):
                            nc.tensor.transpose(
                                pt[:, k, q_sub, ts(j4, P)],
                                zg[:, ti, ts(k, P)], identity[:],
                            )
                    nc.vector.tensor_copy(
                        znt[grp][:, :, 2 * hp : 2 * hp + 2, :], pt[:]
                    )

            # ---- matmul + exp over this group's 2048 columns --------------
            for b in range(NB):
                if S >= 3:
                    pm = psum.tile([P, 4, 512], F32, tag="ps", name=f"pm{grp}_{b}")
                    for k in range(2):
                        for q in range(4):
                            nc.tensor.matmul(
                                pm[:, q, :],
                                zrawT[:, k, ts(b, P)],
                                znt[grp][:, k, q, :],
                                start=(k == 0),
                                stop=(k == 1),
                            )
                if S >= 4:
                    eo = expout.tile([P, 4, 512], MM_DT, tag="eo", bufs=2,
                                     name=f"eo{grp}_{b}")
                    nc.scalar.activation(
                        eo[:], pm[:], AF.Exp,
                        bias=negtwo[:], scale=rnr2[:, b : b + 1],
                        accum_out=denoms[:, b, grp : grp + 1],
                    )

            if grp == 0 and S >= 1:
                # diag term exp(2*selfdot_unit - 2), selfdot = |z|^2 * rnr^2
                sd = small.tile([P, NB], F32)
                nc.vector.tensor_tensor(out=sd[:], in0=sa[:, 0:NB],
                                        in1=rn2[:, 0:NB], op=ALU.mult)
                nc.vector.tensor_tensor(out=sd[:], in0=sd[:], in1=rn2[:, 0:NB],
                                        op=ALU.mult)
                diag = small.tile([P, NB], F32)
                nc.scalar.activation(diag[:], sd[:], AF.Exp, bias=negtwo[:],
                                     scale=SCALE)
                # pos_unit = rawpos * rn_row * rn_pair
                posx = small.tile([P, NB], F32)
                nc.vector.tensor_tensor(out=posx[:], in0=rawpos[:],
                                        in1=rn2[:, 0:NB], op=ALU.mult)
                nc.vector.tensor_tensor(out=posx[:], in0=posx[:],
                                        in1=rn2[:, NB:], op=ALU.mult)

        # ---- epilogue: per-row loss ---------------------------------------
        if S < 4:
            outsb0 = small.tile([NB, P], F32)
            nc.gpsimd.memset(outsb0[:], 0.0)
            nc.sync.dma_start(out[:, :], outsb0[:])
        else:
            denom = small.tile([P, NB], F32)
            nc.vector.tensor_reduce(
                denom[:], denoms[:], axis=mybir.AxisListType.X, op=ALU.add
            )
            nc.vector.tensor_tensor(out=denom[:], in0=denom[:], in1=diag[:],
                                    op=ALU.subtract)
            logd = small.tile([P, NB], F32)
            nc.scalar.activation(logd[:], denom[:], AF.Ln)
            loss = small.tile([P, NB], F32)
            nc.vector.tensor_scalar_mul(loss[:], posx[:], -2.0)
            nc.vector.tensor_tensor(out=loss[:], in0=loss[:], in1=logd[:],
                                    op=ALU.add)
            nc.vector.tensor_scalar_add(loss[:], loss[:], 2.0)

            # transpose [128, 8] -> [8, 128] so the output DMA is contiguous
            pl = psum.tile([P, 4, 512], F32, tag="ps")
            nc.tensor.transpose(pl[0:NB, 0, 0:P], loss[:], identity[:])
            outsb = small.tile([NB, P], F32)
            nc.vector.tensor_copy(outsb[:], pl[0:NB, 0, 0:P])
            nc.sync.dma_start(out[:, :], outsb[:])

    nc.compile()
    return nc


_NC = None


def _get_nc():
    global _NC
    if _NC is None:
        _NC = build_nc()
    return _NC


def _in_maps(z1, z2):
    z1 = np.ascontiguousarray(z1, dtype=np.float32)
    z2 = np.ascontiguousarray(z2, dtype=np.float32)
    z = np.concatenate([z1, z2], axis=0)
    maps = []
    for c in range(NCORES):
        lo = c * RPC
        plo = (lo + N) % R
        maps.append(
            {
                "z1": z1,
                "z2": z2,
                "zrows": np.ascontiguousarray(z[lo : lo + RPC]),
                "zpair": np.ascontiguousarray(z[plo : plo + RPC]),
            }
        )
    return maps


def run(z1, z2, trace=False, **kwargs):
    nc = _get_nc()
    res = run_bass_kernel_spmd(
        nc, _in_maps(z1, z2), list(range(NCORES)), trace=trace, **kwargs
    )
    rows = np.concatenate(
        [np.asarray(res.results[c]["loss_rows"]).reshape(-1) for c in range(NCORES)]
    )
    return np.float32(rows.mean()), res


def kernel(z1, z2):
    loss, _ = run(z1, z2)
    return loss
